# revision 8
# baseline (speedup 1.0000x reference)
"""Trainium2 Bass kernel for CodeGNN message-passing network (8 NeuronCores).

Strategy:
  - Nodes padded 50000 -> 50176 = 8 cores x 49 windows x 128.
  - Edges (plus one self-loop edge per real node) partitioned by dst owner
    core, grouped by 128-node dst window, split into lo/hi halves so
    dma_gather's int16 indices can address the 50176-row table via two
    offset views (lo = rows [0, 32768), hi = rows [17408, 50176)).
  - Per layer: hl = h @ W + b computed on each core's node slice, AllGathered
    into the full-table gather source; message rows are dma_gathered and
    segment-summed on the tensor engine with one-hot indicator matmuls.
  - Edge-feature terms are linear, so they collapse to
    EA_agg @ (edge_w @ We_l) + deg * (edge_b @ We_l + be_l), with EA_agg
    (raw 6-dim edge-attr segment sums + degree) computed once in layer 0
    reusing the same indicator matmuls.
  - Mean-pool via one-hot(batch) matmuls + AllReduce; classifier replicated.
"""

import sys

sys.path.insert(0, "/opt/trn_rl_repo")

import numpy as np

import concourse.bass as bass
import concourse.bacc as bacc
import concourse.mybir as mybir
import concourse.tile as tile
from concourse.bass_utils import run_bass_kernel_spmd

# problem constants (hardcoded per harness contract)
N = 50000
E = 800000
FEAT = 128
HID = 256
EDIM = 6
NGRAPH = 64
NCLS = 2
LAYERS = 4
LN_EPS = 1e-5

NCORES = 8
NP = 50176            # padded nodes = 8 * 6272
PERCORE = 6272        # nodes per core = 49 * 128
NWIN = 49             # 128-node windows per core
W128 = 128
LO_SZ = 32768         # lo view rows [0, 32768)
HI_BASE = 17408       # hi view rows [17408, 50176)
GT = 8                # tiles per dma_gather call (1024 rows)

TRACE = False         # test.py flips this for profiling
DEBUG = False         # adds intermediate-dump outputs

f32 = mybir.dt.float32
i16 = mybir.dt.int16


# ----------------------------------------------------------------------------
# host-side preprocessing (index manipulation + weight folding only)
# ----------------------------------------------------------------------------

def _ceil_div(a, b):
    return -(-a // b)


def _pack_idx16(flat):
    """Pack a per-call index vector (len = k*128, k<=8) into the int16 layout
    dma_gather reads: IDX[p%16, p//16 + 8*g] = flat[g*128 + p], tiled x8."""
    n = len(flat)
    out = np.zeros((16, n // 16), dtype=np.int16)
    e = np.arange(n)
    p, g = e % 128, e // 128
    out[p % 16, p // 16 + 8 * g] = flat
    return np.tile(out, (8, 1))


def _preprocess(x, edge_index, edge_attr, batch, params):
    src = np.asarray(edge_index[0], dtype=np.int64)
    dst = np.asarray(edge_index[1], dtype=np.int64)
    ea = np.asarray(edge_attr, dtype=np.float32)
    batch = np.asarray(batch, dtype=np.int64)
    x = np.asarray(x, dtype=np.float32)

    # append self-loop pseudo-edges (carry hl[v], no edge-attr, no deg count)
    allsrc = np.concatenate([src, np.arange(N, dtype=np.int64)])
    alldst = np.concatenate([dst, np.arange(N, dtype=np.int64)])
    eaid = np.concatenate([np.arange(E, dtype=np.int64),
                           np.full(N, -1, dtype=np.int64)])

    core = alldst // PERCORE
    win = (alldst % PERCORE) // W128
    dloc = alldst % W128

    # ea7: [E] -> 7 cols (6 attrs + ones for degree); self/pad rows are zero
    ea7 = np.zeros((E + 1, 7), dtype=np.float32)
    ea7[:E, :EDIM] = ea
    ea7[:E, EDIM] = 1.0
    # eaid == -1 maps to row E (zeros)

    # per (core, window): lo/hi split with flexible middle band
    percore = []
    cnt_lo = np.zeros((NCORES, NWIN), dtype=np.int64)
    cnt_hi = np.zeros((NCORES, NWIN), dtype=np.int64)
    for c in range(NCORES):
        mc = core == c
        s_c, w_c, dl_c, id_c = allsrc[mc], win[mc], dloc[mc], eaid[mc]
        order = np.argsort(w_c, kind="stable")
        s_c, w_c, dl_c, id_c = s_c[order], w_c[order], dl_c[order], id_c[order]
        wstart = np.searchsorted(w_c, np.arange(NWIN + 1))
        wins = []
        for w in range(NWIN):
            sl = slice(wstart[w], wstart[w + 1])
            s_w, dl_w, id_w = s_c[sl], dl_c[sl], id_c[sl]
            must_lo = s_w < HI_BASE
            must_hi = s_w >= LO_SZ
            mid = ~must_lo & ~must_hi
            n_lo, n_mid = int(must_lo.sum()), int(mid.sum())
            # fill lo up to a multiple of 128 using mid edges (no waste)
            lo_target = min(_ceil_div(n_lo, W128) * W128, n_lo + n_mid)
            take_mid = lo_target - n_lo
            mi = np.nonzero(mid)[0]
            lo_sel = np.zeros(len(s_w), dtype=bool)
            lo_sel[must_lo] = True
            lo_sel[mi[:take_mid]] = True
            hi_sel = ~lo_sel
            wins.append((
                (s_w[lo_sel], dl_w[lo_sel], id_w[lo_sel]),
                (s_w[hi_sel] - HI_BASE, dl_w[hi_sel], id_w[hi_sel]),
            ))
            cnt_lo[c, w] = int(lo_sel.sum())
            cnt_hi[c, w] = int(hi_sel.sum())
        percore.append(wins)

    T_lo = [int(_ceil_div(int(cnt_lo[:, w].max()), W128)) for w in range(NWIN)]
    T_hi = [int(_ceil_div(int(cnt_hi[:, w].max()), W128)) for w in range(NWIN)]
    TLO, THI = sum(T_lo), sum(T_hi)

    def build_stream(c, half):
        """Return (idx int16 [128, 8*T], dstloc f32 [128, T], ea7 f32 [128, 7*T])."""
        T = TLO if half == 0 else THI
        Tw = T_lo if half == 0 else T_hi
        flat_idx = np.zeros(T * W128, dtype=np.int64)
        dl = np.full(T * W128, -1.0, dtype=np.float32)
        eaidx = np.full(T * W128, E, dtype=np.int64)  # -> zero ea7 row
        off = 0
        for w in range(NWIN):
            s_w, dl_w, id_w = percore[c][w][half]
            n = len(s_w)
            flat_idx[off:off + n] = s_w
            dl[off:off + n] = dl_w.astype(np.float32)
            eaidx[off:off + n] = np.where(id_w >= 0, id_w, E)
            off += Tw[w] * W128
        # idx arrays per gather call
        cols = []
        for k in range(_ceil_div(T, GT)):
            t0, t1 = k * GT, min((k + 1) * GT, T)
            cols.append(_pack_idx16(flat_idx[t0 * W128:t1 * W128]))
        idx16 = np.concatenate(cols, axis=1) if cols else np.zeros((128, 0), np.int16)
        dl_arr = dl.reshape(T, W128).T.copy()                     # [128, T]
        ea_arr = ea7[eaidx].reshape(T, W128, 7)                   # [T, 128, 7]
        ea_arr = np.transpose(ea_arr, (1, 0, 2)).reshape(W128, T * 7).copy()
        return idx16.astype(np.int16), dl_arr, ea_arr

    # parameters (shared across cores) ---------------------------------------
    p = {k: np.asarray(v, dtype=np.float32) for k, v in params.items()}

    def pad256(v):
        out = np.zeros(256, dtype=np.float32)
        out[:len(v)] = v
        return out

    brep_rows = [p["enc_b"], p["enc_g"], p["enc_bt"]]
    for l in range(LAYERS):
        bias2 = p[f"l{l}_bs"].copy()
        if l > 0:
            bias2 = bias2 + p[f"sk{l}_b"]
        brep_rows += [p[f"l{l}_b"], bias2, p[f"l{l}_g"], p[f"l{l}_bt"]]
    brep_rows += [p["c1_b"], p["c1_g"], p["c1_bt"],
                  p["c2_b"], p["c2_g"], p["c2_bt"], p["c3_b"]]
    brep = np.stack([pad256(v) for v in brep_rows])               # [26, 256]

    def ktiles(w):  # [fin, fout] -> [128, (fin/128)*fout]
        fin, fout = w.shape
        return np.concatenate([w[k * 128:(k + 1) * 128, :]
                               for k in range(fin // 128)], axis=1)

    wl = [ktiles(p[f"l{l}_W"]) for l in range(LAYERS)]            # [128, 512]
    ws = []
    for l in range(LAYERS):
        wsk = p[f"sk{l}_w"] if l > 0 else np.zeros((HID, HID), np.float32)
        cat = np.concatenate([p[f"l{l}_Ws"], wsk], axis=1)        # [256, 512]
        ws.append(ktiles(cat))                                    # [128, 1024]
    wei = []
    for l in range(LAYERS):
        m = np.zeros((8, 256), dtype=np.float32)
        m[:EDIM] = p["edge_w"] @ p[f"l{l}_We"]
        m[EDIM] = p["edge_b"] @ p[f"l{l}_We"] + p[f"l{l}_be"]
        wei.append(m)
    c1w = ktiles(p["c1_w"])                                       # [128, 512]
    c2w = ktiles(p["c2_w"])                                       # [128, 256]
    c3w = p["c3_w"]                                               # [128, 2]

    consts = np.zeros((128, 320), dtype=np.float32)
    consts[:, :128] = np.arange(128, dtype=np.float32)[None, :]   # iota128
    consts[:, 128:192] = np.arange(64, dtype=np.float32)[None, :] # iota64
    consts[:, 192:320] = np.eye(128, dtype=np.float32)            # identity

    shared = {
        "wenc": p["enc_w"], "brep": brep, "consts": consts,
        "c1w": c1w, "c2w": c2w, "c3w": c3w,
    }
    for l in range(LAYERS):
        shared[f"wl{l}"] = wl[l]
        shared[f"ws{l}"] = ws[l]
        shared[f"wei{l}"] = wei[l]

    in_maps = []
    for c in range(NCORES):
        xs = np.zeros((PERCORE, FEAT), dtype=np.float32)
        lo, hi = c * PERCORE, min((c + 1) * PERCORE, N)
        if hi > lo:
            xs[:hi - lo] = x[lo:hi]
        bl = np.full(PERCORE, -1.0, dtype=np.float32)
        if hi > lo:
            bl[:hi - lo] = batch[lo:hi].astype(np.float32)
        bloc = bl.reshape(NWIN, W128).T.copy()                    # [128, 49]
        ilo, dlo_a, ealo = build_stream(c, 0)
        ihi, dhi_a, eahi = build_stream(c, 1)
        m = dict(shared)
        m.update({"xs": xs, "bloc": bloc,
                  "idxlo": ilo, "idxhi": ihi,
                  "dlo": dlo_a, "dhi": dhi_a,
                  "ealo": ealo, "eahi": eahi})
        in_maps.append(m)

    return in_maps, T_lo, T_hi, TLO, THI


# ----------------------------------------------------------------------------
# device program
# ----------------------------------------------------------------------------

def _build(T_lo, T_hi, TLO, THI):
    nc = bacc.Bacc("TRN2", target_bir_lowering=False, debug=False,
                   num_devices=NCORES)

    # IO ---------------------------------------------------------------------
    xs_d = nc.dram_tensor("xs", [PERCORE, FEAT], f32, kind="ExternalInput")
    wenc_d = nc.dram_tensor("wenc", [FEAT, HID], f32, kind="ExternalInput")
    brep_d = nc.dram_tensor("brep", [26, 256], f32, kind="ExternalInput")
    consts_d = nc.dram_tensor("consts", [128, 320], f32, kind="ExternalInput")
    wl_d = [nc.dram_tensor(f"wl{l}", [128, 512], f32, kind="ExternalInput")
            for l in range(LAYERS)]
    ws_d = [nc.dram_tensor(f"ws{l}", [128, 1024], f32, kind="ExternalInput")
            for l in range(LAYERS)]
    wei_d = [nc.dram_tensor(f"wei{l}", [8, 256], f32, kind="ExternalInput")
             for l in range(LAYERS)]
    c1w_d = nc.dram_tensor("c1w", [128, 512], f32, kind="ExternalInput")
    c2w_d = nc.dram_tensor("c2w", [128, 256], f32, kind="ExternalInput")
    c3w_d = nc.dram_tensor("c3w", [128, 2], f32, kind="ExternalInput")
    bloc_d = nc.dram_tensor("bloc", [128, NWIN], f32, kind="ExternalInput")
    idxlo_d = nc.dram_tensor("idxlo", [128, 8 * TLO], i16, kind="ExternalInput")
    idxhi_d = nc.dram_tensor("idxhi", [128, 8 * THI], i16, kind="ExternalInput")
    dlo_d = nc.dram_tensor("dlo", [128, TLO], f32, kind="ExternalInput")
    dhi_d = nc.dram_tensor("dhi", [128, THI], f32, kind="ExternalInput")
    ealo_d = nc.dram_tensor("ealo", [128, 7 * TLO], f32, kind="ExternalInput")
    eahi_d = nc.dram_tensor("eahi", [128, 7 * THI], f32, kind="ExternalInput")
    logits_d = nc.dram_tensor("logits", [NGRAPH, NCLS], f32,
                              kind="ExternalOutput")
    if DEBUG:
        dbg_h0_d = nc.dram_tensor("dbg_h0", [128, NWIN * HID], f32,
                                  kind="ExternalOutput")
        dbg_h1_d = nc.dram_tensor("dbg_h1", [128, NWIN * HID], f32,
                                  kind="ExternalOutput")
        dbg_ea_d = nc.dram_tensor("dbg_ea", [128, NWIN * 8], f32,
                                  kind="ExternalOutput")
        dbg_hl0_d = nc.dram_tensor("dbg_hl0", [PERCORE, HID], f32,
                                  kind="ExternalOutput")
        dbg_pool_d = nc.dram_tensor("dbg_pool", [NGRAPH, 257], f32,
                                  kind="ExternalOutput")
        dbg_agg_d = nc.dram_tensor("dbg_agg", [128, NWIN * HID], f32,
                                   kind="ExternalOutput")
        dbg_hL_d = [nc.dram_tensor(f"dbg_hL{l}", [128, NWIN * HID], f32,
                                   kind="ExternalOutput")
                    for l in range(LAYERS)]

    # stream bookkeeping (identical on all cores) ----------------------------
    lo_off, hi_off = [0], [0]
    for w in range(NWIN):
        lo_off.append(lo_off[-1] + T_lo[w])
        hi_off.append(hi_off[-1] + T_hi[w])
    ncalls = {0: _ceil_div(TLO, GT), 1: _ceil_div(THI, GT)}
    Ttot = {0: TLO, 1: THI}
    Tmax = max(max(T_lo), max(T_hi), 1)

    def brow(i):  # broadcast-load row i of brep into [128, 256]
        return bass.AP(tensor=brep_d, offset=i * 256,
                       ap=[[0, 128], [1, 256]])

    with tile.TileContext(nc) as tc:
        with (
            tc.tile_pool(name="sg", bufs=1) as sg,       # persistent singles
            tc.tile_pool(name="ld", bufs=3) as ld,       # x / ea7 load tiles
            tc.tile_pool(name="tp", bufs=4) as tp,       # transposed copies
            tc.tile_pool(name="ind", bufs=4) as ind_p,   # indicators
            tc.tile_pool(name="ep", bufs=3) as ep,       # epilogue temps
            tc.tile_pool(name="st", bufs=8) as st,       # small stats tiles
            tc.tile_pool(name="gb", bufs=4) as gb,       # gather buffers
            tc.tile_pool(name="pm", bufs=4, space="PSUM") as pm,
            tc.tile_pool(name="pg", bufs=2, space="PSUM") as pg,
            tc.tile_pool(name="dr", bufs=1, space="DRAM") as dr,
        ):
            # ---- persistent SBUF state
            consts = sg.tile([128, 320], f32, tag="consts")
            nc.sync.dma_start(consts[:], consts_d[:])
            iota128 = consts[:, 0:128]
            iota64 = consts[:, 128:192]
            ident = consts[:, 192:320]

            wenc = sg.tile([128, HID], f32, tag="wenc")
            nc.sync.dma_start(wenc[:], wenc_d[:])
            wl_s = [sg.tile([128, 512], f32, tag=f"wl{l}", name=f"wl{l}s")
                    for l in range(LAYERS)]
            ws_s = [sg.tile([128, 1024], f32, tag=f"ws{l}", name=f"ws{l}s")
                    for l in range(LAYERS)]
            wei_s = [sg.tile([8, 256], f32, tag=f"wei{l}", name=f"wei{l}s")
                     for l in range(LAYERS)]
            for l in range(LAYERS):
                nc.sync.dma_start(wl_s[l][:], wl_d[l][:])
                nc.sync.dma_start(ws_s[l][:], ws_d[l][:])
                nc.sync.dma_start(wei_s[l][:], wei_d[l][:])
            c1w = sg.tile([128, 512], f32, tag="c1w")
            c2w = sg.tile([128, 256], f32, tag="c2w")
            c3w = sg.tile([128, 2], f32, tag="c3w")
            nc.sync.dma_start(c1w[:], c1w_d[:])
            nc.sync.dma_start(c2w[:], c2w_d[:])
            nc.sync.dma_start(c3w[:], c3w_d[:])

            reps = sg.tile([128, 26, 256], f32, tag="reps")
            for i in range(26):
                nc.gpsimd.dma_start(reps[:, i, :], brow(i))
            R_ENCB, R_ENCG, R_ENCBT = 0, 1, 2

            def lrow(l, j):  # j: 0=b, 1=bias2, 2=g, 3=bt
                return reps[:, 3 + 4 * l + j, :]

            R_C1B, R_C1G, R_C1BT, R_C2B, R_C2G, R_C2BT, R_C3B = range(19, 26)

            bloc = sg.tile([128, NWIN], f32, tag="bloc")
            nc.sync.dma_start(bloc[:], bloc_d[:])
            idx_s = {0: sg.tile([128, max(8 * TLO, 1)], i16, tag="idxlo",
                                 name="idxlo_s"),
                     1: sg.tile([128, max(8 * THI, 1)], i16, tag="idxhi",
                                 name="idxhi_s")}
            nc.sync.dma_start(idx_s[0][:, :8 * TLO], idxlo_d[:])
            nc.sync.dma_start(idx_s[1][:, :8 * THI], idxhi_d[:])
            dl_s = {0: sg.tile([128, max(TLO, 1)], f32, tag="dlo", name="dlo_s"),
                    1: sg.tile([128, max(THI, 1)], f32, tag="dhi", name="dhi_s")}
            nc.sync.dma_start(dl_s[0][:, :TLO], dlo_d[:])
            nc.sync.dma_start(dl_s[1][:, :THI], dhi_d[:])

            ea_agg = sg.tile([128, NWIN * 8], f32, tag="ea_agg")
            nc.vector.memset(ea_agg[:], 0.0)
            h = sg.tile([128, NWIN * HID], f32, tag="h")
            ones = sg.tile([128, 1], f32, tag="ones")
            nc.vector.memset(ones[:], 1.0)
            eps_t = sg.tile([128, 1], f32, tag="eps")
            nc.vector.memset(eps_t[:], LN_EPS)

            # ---- DRAM internals
            hl_dram = dr.tile([PERCORE, HID], f32, tag="hl_dram")
            hl_ag = dr.tile([NP, HID], f32, tag="hl_ag")
            pool_in = dr.tile([NGRAPH, 257], f32, tag="pool_in")
            pool_out = dr.tile([NGRAPH, 257], f32, tag="pool_out")

            ea7_d = {0: ealo_d, 1: eahi_d}

            # ---- helpers
            def transpose128(src_ap):
                """[128, <=128] sbuf -> [<=128, 128] transposed sbuf copy."""
                kp = src_ap.shape[0]
                ps = pm.tile([128, 128], f32, tag="pmm")
                nc.tensor.transpose(ps[:src_ap.shape[1], :kp], src_ap,
                                    ident[:kp, :kp])
                out = tp.tile([128, 128], f32, tag="hT")
                nc.scalar.copy(out[:src_ap.shape[1], :kp],
                               ps[:src_ap.shape[1], :kp])
                return out

            def layernorm(ap, pdim, d, g_rep, bt_rep, relu, out_ap):
                """LN along free dim of ap [pdim, d] (in place), then
                *g + bt (+optional relu) into out_ap."""
                stats = st.tile([128, 6], f32, tag="stats")
                mv = st.tile([128, 2], f32, tag="mv")
                nc.vector.bn_stats(out=stats[:pdim], in_=ap)
                nc.vector.bn_aggr(out=mv[:pdim], in_=stats[:pdim])
                rstd = st.tile([128, 1], f32, tag="rstd")
                nc.scalar.activation(out=rstd[:pdim], in_=mv[:pdim, 1:2],
                                     func=mybir.ActivationFunctionType.Sqrt,
                                     bias=eps_t[:pdim], scale=1.0)
                nc.vector.reciprocal(out=rstd[:pdim], in_=rstd[:pdim])
                nc.vector.tensor_scalar(out=ap, in0=ap,
                                        scalar1=mv[:pdim, 0:1],
                                        scalar2=rstd[:pdim],
                                        op0=mybir.AluOpType.subtract,
                                        op1=mybir.AluOpType.mult)
                nc.vector.tensor_mul(out=ap, in0=ap, in1=g_rep[:pdim, :d])
                if relu:
                    nc.vector.tensor_add(out=ap, in0=ap,
                                         in1=bt_rep[:pdim, :d])
                    nc.scalar.activation(out=out_ap, in_=ap,
                                         func=mybir.ActivationFunctionType.Relu)
                else:
                    nc.vector.tensor_tensor(out=out_ap, in0=ap,
                                            in1=bt_rep[:pdim, :d],
                                            op=mybir.AluOpType.add)

            # ---- encoder: h = relu(LN(x @ enc_w + enc_b))
            for w in range(NWIN):
                xt = ld.tile([128, FEAT], f32, tag="xt")
                nc.sync.dma_start(xt[:], xs_d[w * 128:(w + 1) * 128, :])
                xT = transpose128(xt[:])
                pe = pm.tile([128, HID], f32, tag="pmm")
                nc.tensor.matmul(pe[:], lhsT=xT[:], rhs=wenc[:],
                                 start=True, stop=True)
                t = ep.tile([128, HID], f32, tag="ept")
                nc.vector.tensor_add(out=t[:], in0=pe[:],
                                     in1=reps[:, R_ENCB, :])
                layernorm(t[:], 128, HID, reps[:, R_ENCG, :],
                          reps[:, R_ENCBT, :], True,
                          h[:, w * HID:(w + 1) * HID])

            if DEBUG:
                nc.sync.dma_start(dbg_h0_d[:], h[:])

            # ---- layers
            gtiles = {}     # (half, call) -> gather buffer tile
            issued = {}

            def ensure_call(half, k, l):
                if (half, k) in issued:
                    return
                issued[(half, k)] = True
                t0, t1 = k * GT, min((k + 1) * GT, Ttot[half])
                nt = t1 - t0
                nidx = nt * W128
                g = gb.tile([128, GT, HID], f32, tag="g")
                src_view = hl_ag[0:LO_SZ, :] if half == 0 \
                    else hl_ag[HI_BASE:HI_BASE + LO_SZ, :]
                nc.gpsimd.dma_gather(
                    g[:, :nt, :], src_view,
                    idx_s[half][:, 64 * k:64 * k + 8 * nt],
                    nidx, nidx, HID)
                gtiles[(half, k)] = g

            for l in range(LAYERS):
                # phase 1: hl tiles -> DRAM slice
                for w in range(NWIN):
                    hw = h[:, w * HID:(w + 1) * HID]
                    hT0 = transpose128(hw[:, 0:128])
                    hT1 = transpose128(hw[:, 128:256])
                    phl = pm.tile([128, HID], f32, tag="pmm")
                    nc.tensor.matmul(phl[:], lhsT=hT0[:], rhs=wl_s[l][:, 0:256],
                                     start=True, stop=False)
                    nc.tensor.matmul(phl[:], lhsT=hT1[:], rhs=wl_s[l][:, 256:512],
                                     start=False, stop=True)
                    hlt = ep.tile([128, HID], f32, tag="hlt")
                    nc.vector.tensor_add(out=hlt[:], in0=phl[:],
                                         in1=lrow(l, 0))
                    nc.sync.dma_start(hl_dram[w * 128:(w + 1) * 128, :], hlt[:])
                if DEBUG and l == 0:
                    nc.sync.dma_start(dbg_hl0_d[:], hl_dram[:])
                # all-gather hl
                nc.gpsimd.collective_compute(
                    "AllGather", mybir.AluOpType.bypass,
                    replica_groups=[list(range(NCORES))],
                    ins=[hl_dram.opt()], outs=[hl_ag.opt()])

                issued.clear()
                gtiles.clear()

                # phase 2+3: per-window aggregation & update
                for w in range(NWIN):
                    wtiles = ([(0, lo_off[w] + i) for i in range(T_lo[w])]
                              + [(1, hi_off[w] + i) for i in range(T_hi[w])])
                    pagg = pg.tile([128, HID], f32, tag="pagg")
                    if l == 0:
                        pea = pm.tile([128, 8], f32, tag="pmm")
                        ea_ld = {}
                        for half in (0, 1):
                            Tw = T_lo[w] if half == 0 else T_hi[w]
                            o = lo_off[w] if half == 0 else hi_off[w]
                            if Tw:
                                et = ld.tile([128, 7 * Tmax], f32, tag="ea7")
                                nc.sync.dma_start(
                                    et[:, :7 * Tw],
                                    ea7_d[half][:, 7 * o:7 * (o + Tw)])
                                ea_ld[half] = et
                    for j, (half, t) in enumerate(wtiles):
                        k, slot = t // GT, t % GT
                        ensure_call(half, k, l)
                        indt = ind_p.tile([128, 128], f32, tag="ind")
                        nc.vector.tensor_scalar(
                            out=indt[:], in0=iota128,
                            scalar1=dl_s[half][:, t:t + 1], scalar2=None,
                            op0=mybir.AluOpType.is_equal)
                        g = gtiles[(half, k)]
                        nc.tensor.matmul(pagg[:], lhsT=indt[:],
                                         rhs=g[:, slot, :],
                                         start=(j == 0), stop=False,
                                         skip_group_check=True)
                        if l == 0:
                            o = lo_off[w] if half == 0 else hi_off[w]
                            jj = t - o
                            nc.tensor.matmul(
                                pea[:, :7], lhsT=indt[:],
                                rhs=ea_ld[half][:, 7 * jj:7 * jj + 7],
                                start=(j == 0), stop=(j == len(wtiles) - 1),
                                skip_group_check=True)
                    if l == 0:
                        nc.scalar.copy(ea_agg[:, 8 * w:8 * w + 7], pea[:, :7])
                    # edge-feature term: EA_agg(w)^T @ wei_l
                    pet = pm.tile([128, 128], f32, tag="pmm")
                    nc.tensor.transpose(pet[:8, :], ea_agg[:, 8 * w:8 * (w + 1)],
                                        ident[:, :])
                    eat = tp.tile([8, 128], f32, tag="eat")
                    nc.scalar.copy(eat[:], pet[:8, :])
                    nc.tensor.matmul(pagg[:], lhsT=eat[:], rhs=wei_s[l][:],
                                     start=False, stop=True,
                                     skip_group_check=True)
                    # h_self + skip sums
                    hw = h[:, w * HID:(w + 1) * HID]
                    hT0 = transpose128(hw[:, 0:128])
                    hT1 = transpose128(hw[:, 128:256])
                    pb = pm.tile([128, HID], f32, tag="pmm")
                    nc.tensor.matmul(pb[:], lhsT=hT0[:], rhs=ws_s[l][:, 0:256],
                                     start=True, stop=False)
                    nc.tensor.matmul(pb[:], lhsT=hT0[:], rhs=ws_s[l][:, 256:512],
                                     start=False, stop=False)
                    nc.tensor.matmul(pb[:], lhsT=hT1[:], rhs=ws_s[l][:, 512:768],
                                     start=False, stop=False)
                    nc.tensor.matmul(pb[:], lhsT=hT1[:], rhs=ws_s[l][:, 768:1024],
                                     start=False, stop=True)
                    # epilogue
                    t = ep.tile([128, HID], f32, tag="ept")
                    if DEBUG and l == 0:
                        dba = ep.tile([128, HID], f32, tag="dba")
                        nc.vector.tensor_copy(out=dba[:], in_=pagg[:])
                        nc.sync.dma_start(
                            dbg_agg_d[:, w * HID:(w + 1) * HID], dba[:])
                    nc.scalar.activation(out=t[:], in_=pagg[:],
                                         func=mybir.ActivationFunctionType.Relu)
                    nc.vector.tensor_add(out=t[:], in0=t[:], in1=pb[:])
                    nc.vector.tensor_add(out=t[:], in0=t[:], in1=lrow(l, 1))
                    layernorm(t[:], 128, HID, lrow(l, 2), lrow(l, 3),
                              l < LAYERS - 1, hw[:])
                if DEBUG:
                    nc.sync.dma_start(dbg_hL_d[l][:], h[:])

            if DEBUG:
                nc.sync.dma_start(dbg_h1_d[:], h[:])
                nc.sync.dma_start(dbg_ea_d[:], ea_agg[:])

            # ---- mean pool + AllReduce
            pp = pm.tile([128, 256], f32, tag="pmm")
            pc = pm.tile([128, 1], f32, tag="pcnt", bufs=1)
            for w in range(NWIN):
                gind = ind_p.tile([128, 64], f32, tag="gind")
                nc.vector.tensor_scalar(out=gind[:], in0=iota64,
                                        scalar1=bloc[:, w:w + 1], scalar2=None,
                                        op0=mybir.AluOpType.is_equal)
                nc.tensor.matmul(pp[:NGRAPH, 0:HID], lhsT=gind[:],
                                 rhs=h[:, w * HID:(w + 1) * HID],
                                 start=(w == 0), stop=(w == NWIN - 1),
                                 skip_group_check=True)
                nc.tensor.matmul(pc[:NGRAPH, 0:1], lhsT=gind[:],
                                 rhs=ones[:],
                                 start=(w == 0), stop=(w == NWIN - 1),
                                 skip_group_check=True)
            pooled = ep.tile([128, 257], f32, tag="pooled")
            nc.scalar.copy(pooled[:NGRAPH, :HID], pp[:NGRAPH, :])
            nc.scalar.copy(pooled[:NGRAPH, HID:257], pc[:NGRAPH, :])
            nc.sync.dma_start(pool_in[:], pooled[:NGRAPH, :])
            nc.gpsimd.collective_compute(
                "AllReduce", mybir.AluOpType.add,
                replica_groups=[list(range(NCORES))],
                ins=[pool_in.opt()], outs=[pool_out.opt()])
            nc.sync.dma_start(pooled[:NGRAPH, :], pool_out[:])
            if DEBUG:
                nc.sync.dma_start(dbg_pool_d[:], pooled[:NGRAPH, :])
            cnt = st.tile([128, 1], f32, tag="cnt")
            nc.vector.tensor_scalar(out=cnt[:NGRAPH],
                                    in0=pooled[:NGRAPH, 256:257],
                                    scalar1=1.0, scalar2=None,
                                    op0=mybir.AluOpType.max)
            nc.vector.reciprocal(out=cnt[:NGRAPH], in_=cnt[:NGRAPH])
            g0 = ep.tile([128, HID], f32, tag="ept")
            nc.vector.tensor_scalar(out=g0[:NGRAPH, :],
                                    in0=pooled[:NGRAPH, :HID],
                                    scalar1=cnt[:NGRAPH], scalar2=None,
                                    op0=mybir.AluOpType.mult)

            # ---- classifier (replicated)
            def dense64(src, fin, w_s, fout, b_rep):
                ps = pm.tile([128, 256], f32, tag="pmm")
                nk = fin // 128
                for k in range(nk):
                    sT = transpose128(src[:NGRAPH, k * 128:(k + 1) * 128])
                    nc.tensor.matmul(ps[:NGRAPH, :fout], lhsT=sT[:, :NGRAPH],
                                     rhs=w_s[:, k * fout:(k + 1) * fout],
                                     start=(k == 0), stop=(k == nk - 1))
                t = ep.tile([128, 256], f32, tag="ept")
                nc.vector.tensor_add(out=t[:NGRAPH, :fout], in0=ps[:NGRAPH, :fout],
                                     in1=b_rep[:NGRAPH, :fout])
                return t

            t1 = dense64(g0, HID, c1w, HID, reps[:, R_C1B, :])
            c1t = ep.tile([128, HID], f32, tag="c1t")
            layernorm(t1[:NGRAPH, :HID], NGRAPH, HID,
                      reps[:, R_C1G, :], reps[:, R_C1BT, :], True,
                      c1t[:NGRAPH, :HID])
            t2 = dense64(c1t, HID, c2w, 128, reps[:, R_C2B, :])
            c2t = ep.tile([128, 128], f32, tag="c2t")
            layernorm(t2[:NGRAPH, :128], NGRAPH, 128,
                      reps[:, R_C2G, :], reps[:, R_C2BT, :], True,
                      c2t[:NGRAPH, :128])
            ps3 = pm.tile([128, 2], f32, tag="pmm")
            c2T = transpose128(c2t[:NGRAPH, :128])
            nc.tensor.matmul(ps3[:NGRAPH, :], lhsT=c2T[:, :NGRAPH], rhs=c3w[:],
                             start=True, stop=True)
            lg = ep.tile([128, 2], f32, tag="lg")
            nc.vector.tensor_add(out=lg[:NGRAPH, :], in0=ps3[:NGRAPH, :],
                                 in1=reps[:NGRAPH, R_C3B, :2])
            nc.sync.dma_start(logits_d[:], lg[:NGRAPH, :])

    nc.compile()
    return nc



def kernel(x, edge_index, edge_attr, batch, params):
    in_maps, T_lo, T_hi, TLO, THI = _preprocess(x, edge_index, edge_attr,
                                                batch, params)
    nc = _build(T_lo, T_hi, TLO, THI)
    res = run_bass_kernel_spmd(nc, in_maps, core_ids=list(range(NCORES)),
                               trace=TRACE)
    kernel.last_result = res
    return res.results[0]["logits"]
